# revision 23
# baseline (speedup 1.0000x reference)
"""GRUCell fused kernel for Trainium2, data-parallel over 8 NeuronCores.

Strategy (v10, all-fp8):
  - Shard batch (16384) across 8 cores -> 2048 rows/core; replicate weights.
  - ALL four gate matmuls (r, z, hg, ig) run as fp8e4 DoubleRow: acts x16,
    weights x512; 192 DR matmuls x ~216ns = ~41.5us PE stream per core.
    Numpy model of the whole scheme: 1.906e-2 rel Fro vs the 2e-2 budget
    (model matched HW to 8e-5 on v9).
  - Scaled-domain epilogue for every j-tile (v9 only did jt0): the 8192x
    PSUM scale rides through m' = (hg_ps + S*b_h)*r and
    s' = (ig_ps + S*b_i) + m' (both DVE STT ops reading PSUM directly),
    unscaled only inside tanh's free affine.  This removes the ACT
    identity-drain: ACT does exactly 3 ops/elem (sig r, sig z, tanh).
  - PSUM as 4 pair-tiles [128,1024] (2 banks each): ring A = r -> hg -> z,
    ring B = ig.  Epilogue ops run at N=1024 (halves the per-op fixed
    cost); r-sigmoids run at N=512 bank granularity so the hg matmuls
    never wait on a pair-wide drain.
  - DMA issue split across queues: sync carries the acts (x8/h8/hb) and
    all outputs; scalar carries the jt0 weights + bias (done before ACT's
    real work starts); gpsimd carries the jt1-3 bulk weights.
  - PE warmup matmuls on a zeroed tile release the HAM clock-gate before
    the first data lands; last j-tile's z-phase drains at N=512 so the
    post-stream tail is one short chain.
"""

import os
import numpy as np
import ml_dtypes
from contextlib import ExitStack

import concourse.bass as bass
import concourse.tile as tile
from concourse import bacc, mybir
from concourse.bass_utils import run_bass_kernel_spmd

B, I, H = 16384, 512, 512
NCORES = 8
BL = B // NCORES          # 2048 rows per core
NB = 512                  # matmul moving free dim (one PSUM bank fp32)
PAIR = 1024               # epilogue tile free dim (two PSUM banks)
NP = BL // PAIR           # 2 pairs per core
P = 128                   # partitions
KT = I // P               # 4 k-subtiles per of x/h
KS = (I + H) // P         # 8 k-subtiles across the r/z contraction
JT = H // P               # 4 output j-tiles per gate

ASCALE = 16.0             # fp8 activation scale
WSCALE = 512.0            # fp8 weight scale
S = ASCALE * WSCALE
INV_SCALE = 1.0 / S

FP32 = mybir.dt.float32
BF16 = mybir.dt.bfloat16
FP8 = mybir.dt.float8e4

_cache = {}


def build_gru_bass():
    """Build (once) the SPMD Bass program for one core's shard."""
    if "nc" in _cache:
        return _cache["nc"]

    nc = bacc.Bacc(
        "TRN2",
        target_bir_lowering=False,
        debug=False,
        enable_asserts=False,
        num_devices=NCORES,
    )

    # feature-major activations, k-subtile-packed: [p, kt, b].
    # One HWDGE ring sustains only ~200 GB/s, so x8 rides the sync ring
    # while h8 rides the scalar (qActDynamicHW) ring in parallel.
    x8 = nc.dram_tensor("x8", [P, KT, BL], FP8, kind="ExternalInput").ap()
    h8 = nc.dram_tensor("h8", [P, KT, BL], FP8, kind="ExternalInput").ap()
    hb = nc.dram_tensor("hb", [P, KT, BL], BF16, kind="ExternalInput").ap()
    # fp8 DoubleRow weights: [(jt,) p, ks, m] = W.T[ks*128+p, jt*128+m]*512
    wr0 = nc.dram_tensor("wr0", [P, KS, P], FP8, kind="ExternalInput").ap()
    wz0 = nc.dram_tensor("wz0", [P, KS, P], FP8, kind="ExternalInput").ap()
    wh0 = nc.dram_tensor("wh0", [P, KT, P], FP8, kind="ExternalInput").ap()
    wi0 = nc.dram_tensor("wi0", [P, KT, P], FP8, kind="ExternalInput").ap()
    wr123 = nc.dram_tensor("wr123", [P, 3, KS, P], FP8, kind="ExternalInput").ap()
    wz123 = nc.dram_tensor("wz123", [P, 3, KS, P], FP8, kind="ExternalInput").ap()
    wh123 = nc.dram_tensor("wh123", [P, 3, KT, P], FP8, kind="ExternalInput").ap()
    wi123 = nc.dram_tensor("wi123", [P, 3, KT, P], FP8, kind="ExternalInput").ap()
    # bias columns: 0..3 b_r per j-tile, 4..7 b_z, 8..11 S*b_h, 12..15 S*b_i
    bias = nc.dram_tensor("bias", [P, 16], FP32, kind="ExternalInput").ap()
    outT = nc.dram_tensor("outT", [H, BL], BF16, kind="ExternalOutput").ap()

    ADD = mybir.AluOpType.add
    MULT = mybir.AluOpType.mult
    SUB = mybir.AluOpType.subtract
    SIG = mybir.ActivationFunctionType.Sigmoid
    TANH = mybir.ActivationFunctionType.Tanh
    DR = mybir.MatmulPerfMode.DoubleRow

    with tile.TileContext(nc) as tc, ExitStack() as ctx:
        wpool = ctx.enter_context(tc.tile_pool(name="weights", bufs=1))
        apool = ctx.enter_context(tc.tile_pool(name="acts", bufs=1))
        ppool = ctx.enter_context(tc.tile_pool(name="psum", bufs=1, space="PSUM"))
        epool = ctx.enter_context(tc.tile_pool(name="epi", bufs=2))

        # PE warmup: matmuls on a zeroed tile, no DMA dependency, so the
        # HAM clock-gate releases to 2.4 GHz before real data arrives.
        # psB0 is first reused by ig-jt0, well after the warmup ends.
        warm = apool.tile([P, NB], BF16, tag="warm", name="warm")
        nc.gpsimd.memset(warm[:], 0.0)
        # Dummy activation BEFORE any scalar-ring DMA issue: walrus puts
        # the ACT_TABLE_LOAD right before the first ACTIVATE, and the
        # table load is itself a DMA on the act ring — force it through
        # while the ring is empty.
        warm_act = epool.tile([P, 1], BF16, tag="warm_act", name="warm_act")
        nc.scalar.activation(out=warm_act[:], in_=warm[:, 0:1],
                             func=mybir.ActivationFunctionType.Sigmoid)
        warm_ps = ppool.tile([P, PAIR], FP32, tag="psB0", name="warm_ps")
        for _ in range(12):
            nc.tensor.matmul(out=warm_ps[:, 0:NB], lhsT=warm[:, 0:P],
                             rhs=warm[:], start=True, stop=True)

        # ---- input DMAs: ONE queue (sync), strict first-use order.
        # The sync HWDGE ring completes FIFO, so a single ring in need
        # order beats parallel rings (which share HBM bandwidth and let
        # low-priority bulk steal it from the critical path). ----
        bias_s = wpool.tile([P, 16], FP32, tag="bias", name="bias_s")
        wr0_s = wpool.tile([P, KS, P], FP8, tag="wr0", name="wr0_s")
        wz0_s = wpool.tile([P, KS, P], FP8, tag="wz0", name="wz0_s")
        wh0_s = wpool.tile([P, KT, P], FP8, tag="wh0", name="wh0_s")
        wi0_s = wpool.tile([P, KT, P], FP8, tag="wi0", name="wi0_s")
        wr123_s = wpool.tile([P, 3, KS, P], FP8, tag="wr123", name="wr123_s")
        wz123_s = wpool.tile([P, 3, KS, P], FP8, tag="wz123", name="wz123_s")
        wh123_s = wpool.tile([P, 3, KT, P], FP8, tag="wh123", name="wh123_s")
        wi123_s = wpool.tile([P, 3, KT, P], FP8, tag="wi123", name="wi123_s")
        x8_s = apool.tile([P, KT, BL], FP8, tag="x8", name="x8_s")
        h8_s = apool.tile([P, KT, BL], FP8, tag="h8", name="h8_s")
        hb_s = apool.tile([P, KT, BL], BF16, tag="hb", name="hb_s")

        # Two HWDGE rings in parallel sum to ~310 GB/s (one alone ~200).
        # The critical acts (x8, h8) lead each ring so both 1MB tensors
        # land ~14us together; everything else follows in need order.
        nc.sync.dma_start(out=wr0_s[:], in_=wr0[:, :, :])
        nc.sync.dma_start(out=bias_s[:], in_=bias[:, :])
        nc.sync.dma_start(out=x8_s[:], in_=x8[:, :, :])
        nc.sync.dma_start(out=wr123_s[:], in_=wr123[:, :, :, :])
        nc.sync.dma_start(out=wz123_s[:], in_=wz123[:, :, :, :])
        nc.sync.dma_start(out=hb_s[:, 1:4, :], in_=hb[:, 1:4, :])
        nc.scalar.dma_start(out=h8_s[:], in_=h8[:, :, :])
        nc.scalar.dma_start(out=wh0_s[:], in_=wh0[:, :, :])
        nc.scalar.dma_start(out=wi0_s[:], in_=wi0[:, :, :])
        nc.scalar.dma_start(out=wz0_s[:], in_=wz0[:, :, :])
        nc.scalar.dma_start(out=hb_s[:, 0, :], in_=hb[:, 0, :])
        nc.scalar.dma_start(out=wh123_s[:], in_=wh123[:, :, :, :])
        nc.scalar.dma_start(out=wi123_s[:], in_=wi123[:, :, :, :])

        # weight slice accessors (jt0 tiles vs merged jt1-3 tiles)
        def wsl(w0, w123):
            def f(jt, ws, n):
                if jt == 0:
                    return w0[:, ws:ws + n, :]
                return w123[:, jt - 1, ws:ws + n, :]
            return f

        wr_sl = wsl(wr0_s, wr123_s)
        wz_sl = wsl(wz0_s, wz123_s)
        wh_sl = wsl(wh0_s, wh123_s)
        wi_sl = wsl(wi0_s, wi123_s)

        # DoubleRow chunks: (acts tile, acts ks, weight ks)
        RZ_CHUNKS = [(x8_s, 0, 0), (x8_s, 2, 2), (h8_s, 0, 4), (h8_s, 2, 6)]
        HG_CHUNKS = [(h8_s, 0, 0), (h8_s, 2, 2)]
        IG_CHUNKS = [(x8_s, 0, 0), (x8_s, 2, 2)]

        def dr_half(ps_pair, p, half, w_sl, jt, chunks):
            """Accumulate one N=512 half of a pair tile over all K chunks."""
            c0 = p * PAIR + half * NB
            nck = len(chunks)
            for kc in range(nck):
                act, ks, ws = chunks[kc]
                nc.tensor.matmul(
                    out=ps_pair[:, half * NB:(half + 1) * NB],
                    lhsT=w_sl(jt, ws, 2),
                    rhs=act[:, ks:ks + 2, c0:c0 + NB],
                    start=(kc == 0), stop=(kc == nck - 1),
                    perf_mode=DR)

        # ---- main loop over output j-tiles ----
        # Emission order is tuned so every engine's FIFO matches data
        # readiness (the ACT queue especially: rsig x4, tanh0, zsig0,
        # zsig1, tanh1 — a jt's late drains must not block the next jt's
        # r-sigmoids).
        for jt in range(JT):
            j0 = jt * P
            last_jt = jt == JT - 1
            # The Tile scheduler re-derives queue order from its own cost
            # model (which runs DR matmuls 2x too fast), pulling
            # drain-dependent matmuls too early.  Pin the phase order with
            # sim-time floors: the scheduler won't schedule an instruction
            # before its floor, and floors are huge (ms) vs its ns clock.
            pc = jt * 6.0

            # phase r: banks A, drained at N=512 bank granularity
            r_ps = [ppool.tile([P, PAIR], FP32, tag=f"psA{p}",
                               name=f"r_ps_{jt}_{p}") for p in range(NP)]
            r_s = [epool.tile([P, PAIR], BF16, tag=f"r_s{p}",
                              name=f"r_s_{jt}_{p}") for p in range(NP)]
            with tc.tile_wait_until(pc):
                for p in range(NP):
                    for half in range(2):
                        dr_half(r_ps[p], p, half, wr_sl, jt, RZ_CHUNKS)
                        nc.scalar.activation(
                            out=r_s[p][:, half * NB:(half + 1) * NB],
                            in_=r_ps[p][:, half * NB:(half + 1) * NB],
                            func=SIG, bias=bias_s[:, jt:jt + 1],
                            scale=INV_SCALE)

            # For jt0-2, the ig matmuls (banks B, no drain dependencies)
            # run BETWEEN the r and hg phases: even when the scheduler
            # pulls the next phase's matmuls early, the hg matmuls (which
            # wait on r-sigmoid PSUM drains) sit ~1.7us after the r STOPs,
            # so the drain latency never stalls the PE.  The last jt keeps
            # hg-before-ig, which shortens its epilogue tail.
            ig_ps = [ppool.tile([P, PAIR], FP32, tag=f"psB{p}",
                                name=f"ig_ps_{jt}_{p}") for p in range(NP)]
            hg_ps = [ppool.tile([P, PAIR], FP32, tag=f"psA{p}",
                                name=f"hg_ps_{jt}_{p}") for p in range(NP)]
            m = [None] * NP

            def ig_mms():
                for p in range(NP):
                    for half in range(2):
                        dr_half(ig_ps[p], p, half, wi_sl, jt, IG_CHUNKS)

            def hg_mms_m():
                for p in range(NP):
                    for half in range(2):
                        dr_half(hg_ps[p], p, half, wh_sl, jt, HG_CHUNKS)
                    m[p] = epool.tile([P, PAIR], BF16, tag=f"m{p}",
                                      name=f"m_{jt}_{p}")
                    nc.vector.scalar_tensor_tensor(
                        out=m[p][:], in0=hg_ps[p][:],
                        scalar=bias_s[:, 8 + jt:9 + jt],
                        in1=r_s[p][:], op0=ADD, op1=MULT)

            if last_jt:
                with tc.tile_wait_until(pc + 1):
                    hg_mms_m()
                with tc.tile_wait_until(pc + 2):
                    ig_mms()
            else:
                with tc.tile_wait_until(pc + 1):
                    ig_mms()
                with tc.tile_wait_until(pc + 2):
                    hg_mms_m()
            s = [None] * NP
            n = [None] * NP
            d = [None] * NP
            with tc.tile_wait_until(pc + 3):
                for p in range(NP):
                    s[p] = epool.tile([P, PAIR], BF16, tag=f"s{p}",
                                      name=f"s_{jt}_{p}")
                    nc.vector.scalar_tensor_tensor(
                        out=s[p][:], in0=ig_ps[p][:],
                        scalar=bias_s[:, 12 + jt:13 + jt],
                        in1=m[p][:], op0=ADD, op1=ADD)

            def tanh_d(p):
                n[p] = epool.tile([P, PAIR], BF16, tag=f"n{p}",
                                  name=f"n_{jt}_{p}")
                nc.scalar.activation(out=n[p][:], in_=s[p][:], func=TANH,
                                     scale=INV_SCALE)
                d[p] = epool.tile([P, PAIR], BF16, tag=f"d{p}",
                                  name=f"d_{jt}_{p}")
                nc.vector.tensor_tensor(
                    out=d[p][:], in0=hb_s[:, jt, p * PAIR:(p + 1) * PAIR],
                    in1=n[p][:], op=SUB)

            # phase z: banks A; z = sig(z_ps/S + b_z), e = z*d, o = n + e.
            z_ps = [ppool.tile([P, PAIR], FP32, tag=f"psA{p}",
                               name=f"z_ps_{jt}_{p}") for p in range(NP)]
            with tc.tile_wait_until(pc + 3):
                tanh_d(0)
            with tc.tile_wait_until(pc + 4):
                for half in range(2):
                    dr_half(z_ps[0], 0, half, wz_sl, jt, RZ_CHUNKS)
                _z_epi(nc, tc, epool, z_ps, bias_s, n, d, outT,
                       jt, j0, 0, 0, PAIR)
            if last_jt:
                # tanh1 before the fine z drains (its d feeds their e's);
                # last pair drains at N=512 to shorten the tail
                with tc.tile_wait_until(pc + 4):
                    tanh_d(1)
                with tc.tile_wait_until(pc + 5):
                    for half in range(2):
                        dr_half(z_ps[1], 1, half, wz_sl, jt, RZ_CHUNKS)
                        _z_epi(nc, tc, epool, z_ps, bias_s, n, d, outT,
                               jt, j0, 1, half, NB)
            else:
                with tc.tile_wait_until(pc + 5):
                    for half in range(2):
                        dr_half(z_ps[1], 1, half, wz_sl, jt, RZ_CHUNKS)
                    # zsig1 ahead of tanh1 in the ACT FIFO (frees psA1 for
                    # the next jt's r); e1/o1 follow tanh1's d on the DVE
                    z_s1 = epool.tile([P, PAIR], BF16, tag="z_s1p",
                                      name=f"z_s_{jt}_1")
                    nc.scalar.activation(out=z_s1[:], in_=z_ps[1][:],
                                         func=SIG,
                                         bias=bias_s[:, 4 + jt:5 + jt],
                                         scale=INV_SCALE)
                    tanh_d(1)
                    e1 = epool.tile([P, PAIR], BF16, tag="e1p",
                                    name=f"e_{jt}_1")
                    nc.vector.tensor_tensor(out=e1[:], in0=z_s1[:],
                                            in1=d[1][:], op=MULT)
                    o1 = epool.tile([P, PAIR], BF16, tag="o1p",
                                    name=f"o_{jt}_1")
                    nc.vector.tensor_tensor(out=o1[:], in0=n[1][:],
                                            in1=e1[:], op=ADD)
                    nc.sync.dma_start(out=outT[j0:j0 + P, PAIR:2 * PAIR],
                                      in_=o1[:])

    nc.compile()
    _cache["nc"] = nc
    return nc


def _z_epi(nc, tc, epool, z_ps, bias_s, n, d, outT, jt, j0, p, half, width):
    """z-sigmoid + e + o + output DMA for a [P, width] slice of pair p."""
    ADD = mybir.AluOpType.add
    MULT = mybir.AluOpType.mult
    SIG = mybir.ActivationFunctionType.Sigmoid
    sl = slice(half * width, (half + 1) * width)
    z_s = epool.tile([P, width], BF16, tag=f"z_s{p}_{half}_{width}",
                     name=f"z_s_{jt}_{p}_{half}")
    nc.scalar.activation(out=z_s[:], in_=z_ps[p][:, sl], func=SIG,
                         bias=bias_s[:, 4 + jt:5 + jt], scale=INV_SCALE)
    e = epool.tile([P, width], BF16, tag=f"e{p}_{half}_{width}",
                   name=f"e_{jt}_{p}_{half}")
    nc.vector.tensor_tensor(out=e[:], in0=z_s[:], in1=d[p][:, sl], op=MULT)
    o = epool.tile([P, width], BF16, tag=f"o{p}_{half}_{width}",
                   name=f"o_{jt}_{p}_{half}")
    nc.vector.tensor_tensor(out=o[:], in0=n[p][:, sl], in1=e[:], op=ADD)
    b0 = p * PAIR + half * width
    nc.sync.dma_start(out=outT[j0:j0 + P, b0:b0 + width], in_=o[:])


def _pack_weights(W_gate, b_gate, W_i, b_i, W_h, b_h):
    fp8 = ml_dtypes.float8_e4m3

    def pack_fp8(WT):   # [K, 512] -> [JT, P, K/128, P]
        ks = WT.shape[0] // P
        a = np.clip(WT * WSCALE, -240.0, 240.0)
        a = a.reshape(ks, P, JT, P).transpose(2, 1, 0, 3)
        return np.ascontiguousarray(a.astype(fp8))

    wr = pack_fp8(W_gate[:H].T)
    wz = pack_fp8(W_gate[H:].T)
    wh = pack_fp8(W_h.T)
    wi = pack_fp8(W_i.T)
    biasp = np.concatenate([
        b_gate[:H].reshape(JT, P).T,
        b_gate[H:].reshape(JT, P).T,
        S * b_h.reshape(JT, P).T,
        S * b_i.reshape(JT, P).T,
    ], axis=1).astype(np.float32)

    def split(w):  # [JT, ...] -> jt0 [P, ...] and jt1-3 [P, 3, ...]
        w0 = np.ascontiguousarray(w[0])
        w123 = np.ascontiguousarray(np.moveaxis(w[1:], 0, 1))
        return w0, w123

    wr0, wr123 = split(wr)
    wz0, wz123 = split(wz)
    wh0, wh123 = split(wh)
    wi0, wi123 = split(wi)
    return (wr0, wr123, wz0, wz123, wh0, wh123, wi0, wi123,
            np.ascontiguousarray(biasp))


def kernel(input, hidden, W_gate, b_gate, W_i, b_i, W_h, b_h):
    input = np.asarray(input, dtype=np.float32)
    hidden = np.asarray(hidden, dtype=np.float32)
    W_gate = np.asarray(W_gate, dtype=np.float32)
    b_gate = np.asarray(b_gate, dtype=np.float32)
    W_i = np.asarray(W_i, dtype=np.float32)
    b_i = np.asarray(b_i, dtype=np.float32)
    W_h = np.asarray(W_h, dtype=np.float32)
    b_h = np.asarray(b_h, dtype=np.float32)

    nc = build_gru_bass()
    (wr0, wr123, wz0, wz123, wh0, wh123, wi0, wi123,
     biasp) = _pack_weights(W_gate, b_gate, W_i, b_i, W_h, b_h)

    bf16 = ml_dtypes.bfloat16
    fp8 = ml_dtypes.float8_e4m3

    def pack8(aT):  # [512, BL] fp32 -> [P, 4, BL] fp8 (scaled)
        a = np.clip(aT * ASCALE, -240.0, 240.0)
        a = a.reshape(KT, P, BL).transpose(1, 0, 2)
        return np.ascontiguousarray(a.astype(fp8))

    def packb(aT):  # [512, BL] fp32 -> [P, KT, BL] bf16
        a = aT.reshape(KT, P, BL).transpose(1, 0, 2)
        return np.ascontiguousarray(a.astype(bf16))

    in_maps = []
    for c in range(NCORES):
        sl = slice(c * BL, (c + 1) * BL)
        xT = np.ascontiguousarray(input[sl].T)
        hT = np.ascontiguousarray(hidden[sl].T)
        in_maps.append({
            "x8": pack8(xT),
            "h8": pack8(hT),
            "hb": packb(hT),
            "wr0": wr0, "wr123": wr123,
            "wz0": wz0, "wz123": wz123,
            "wh0": wh0, "wh123": wh123,
            "wi0": wi0, "wi123": wi123,
            "bias": biasp,
        })

    res = run_bass_kernel_spmd(
        nc, in_maps, list(range(NCORES)),
        trace=bool(int(os.environ.get("GRU_TRACE", "0"))),
    )
    out = np.empty((B, H), dtype=np.float32)
    for c in range(NCORES):
        out[c * BL:(c + 1) * BL, :] = res.results[c]["outT"].astype(np.float32).T
    if res.exec_time_ns is not None:
        kernel.last_exec_time_ns = res.exec_time_ns
        kernel.last_results = res
    return out


kernel.last_exec_time_ns = None
kernel.last_results = None
